# revision 26
# baseline (speedup 1.0000x reference)
"""CRLLoss (majority-masked mean CE) on 8 trn2 NeuronCores — v8 (PE reduce,
3-way exp split).

All-fp8 upload in class-major layout (classes on 125 SBUF partitions x 8
tiles, c = t*125 + p); rows padded to NSG supergroups of 128. Per supergroup
the device computes exp elementwise and reduces over classes with one-hot
stationary matmuls into PSUM, then Ln + keep-masked accumulation per PSUM
region. The loss denominator and the gathered x[label] sum are host-side
(the gather already is).

Three exp paths, balanced so every engine finishes with the DMA stream:
  A: ScalarE activation exp fp8->fp8, 4 DoubleRow fp8 matmuls (PSUM
     partitions [0:32) — DoubleRow cannot use output quadrant packing).
  F: DVE Schraudolph fp8->int16 (y = round(1477.32x + 15300.7), bitcast
     fp16), 8 fp16 one-hot matmuls into partitions [32:96).
  P: same as F but the tensor_scalar runs on GPSIMD (bit-identical on HW).

Emission: first 4 groups [A,F,A,F] with per-group DMAs (engine ramp-up),
then homogeneous runs of 4 groups (A/F/P Bresenham-interleaved, one DMA per
2 runs), last run per-group again to shorten the tail.
"""

import numpy as np
import ml_dtypes

import concourse.bass as bass
import concourse.tile as tile
from concourse import bacc, mybir
from concourse.bass_utils import run_bass_kernel_spmd

LOSS_WEIGHT = 1.0

N, C = 262144, 1000
NCORES = 8
NP_ = 125
NT = 8
M = 128
A_SCH = 1477.3196
B_SCH = 15300.7
CLIP = 5.5

_F32 = mybir.dt.float32
_F16 = mybir.dt.float16
_F8 = mybir.dt.float8e4
_I16 = mybir.dt.int16

_cached = {}
_cached_nc = None


def _pick_an_pn(nsg):
    a_n = max(8, min(92, int(round(nsg * 0.36 / 4)) * 4, nsg - 8))
    return a_n, 0


def _sched(nsg, a_n, p_n):
    """units [(type, ngroups)]; types[g]; info[g] = (region, q); regions."""
    assert nsg % 4 == 0 and nsg >= 32 and a_n % 4 == 0 and p_n % 4 == 0
    f_n = nsg - a_n - p_n
    assert 8 <= a_n <= 96 and (f_n + p_n) <= 192 and f_n >= 8

    n_mid = (nsg - 16) // 4
    aruns = (a_n - 8) // 4
    pruns = p_n // 4
    nb_a = n_mid - 2           # no A-units in the last 2 mid positions
    mid = []
    ca = 0
    for i in range(n_mid):
        if i < nb_a and ((i + 1) * aruns) // nb_a > ca:
            mid.append('A')
            ca += 1
        else:
            mid.append('F')
    # place P among F positions, excluding the last few (Pool latency)
    fpos = [i for i, t in enumerate(mid) if t == 'F' and i < n_mid - 4]
    cp = 0
    for j in range(len(fpos)):
        if ((j + 1) * pruns) // len(fpos) > cp:
            mid[fpos[j]] = 'P'
            cp += 1

    units = [('A', 1), ('F', 1), ('A', 1), ('F', 1)]
    units += [(t, 4) for t in mid]
    units += [('A', 2), ('F', 2), ('A', 2), ('F', 2),
              ('A', 1), ('F', 1), ('A', 1), ('F', 1)]

    types = []
    for t, n in units:
        types += [t] * n
    assert len(types) == nsg and types.count('A') == a_n

    regions = []
    for b in range(3):
        regions.append(dict(kind='A', bank=b, qb=0,
                            nslots=max(0, min(32, a_n - 32 * b))))
    fp_n = f_n + p_n
    for rf in range(6):
        regions.append(dict(kind='F', bank=rf // 2, qb=32 * (1 + rf % 2),
                            nslots=max(0, min(32, fp_n - 32 * rf))))

    info = []
    ai = fi = 0
    for g in range(nsg):
        if types[g] == 'A':
            info.append((ai // 32, ai % 32))
            ai += 1
        else:
            info.append((3 + fi // 32, fi % 32))
            fi += 1
    return units, types, info, regions


def _build_nc(nsg, a_n, p_n):
    units, types, info, regions = _sched(nsg, a_n, p_n)
    nc = bacc.Bacc("TRN2", debug=False, target_bir_lowering=False)

    x = nc.dram_tensor("x", [NP_, nsg * 1024], _F8, kind="ExternalInput")
    w8 = nc.dram_tensor("w8", [NP_, 32, 2, 32], _F8, kind="ExternalInput")
    w16 = nc.dram_tensor("w16", [NP_, 32, 32], _F16, kind="ExternalInput")
    keepf = nc.dram_tensor("keepf", [96, 384], _F32, kind="ExternalInput")
    out = nc.dram_tensor("out", [96, 3], _F32, kind="ExternalOutput")

    mm_total = [regions[r]['nslots'] * (4 if regions[r]['kind'] == 'A' else 8)
                for r in range(9)]
    mm_done = [0] * 9

    with tile.TileContext(nc) as tc:
        with (
            tc.tile_pool(name="axp", bufs=5) as axp,
            tc.tile_pool(name="fxp", bufs=5) as fxp,
            tc.tile_pool(name="pxp", bufs=6) as pxp,
            tc.tile_pool(name="e8p", bufs=4) as e8p,
            tc.tile_pool(name="btp", bufs=3) as btp,
            tc.tile_pool(name="ptp", bufs=6) as ptp,
            tc.tile_pool(name="consts", bufs=1) as consts,
            tc.tile_pool(name="ps", bufs=1, space="PSUM") as ps,
        ):
            nc.scalar.add_instruction(mybir.InstLoadActFuncSet(
                name=nc.get_next_instruction_name(), ins=[], outs=[],
                act_func_set_id=6))

            w8t = consts.tile([NP_, 32, 2, 32], _F8)
            w16t = consts.tile([NP_, 32, 32], _F16)
            keep_s = consts.tile([96, 384], _F32)
            logz = consts.tile([96, 384], _F32)
            dum = consts.tile([96, 384], _F32)
            out_t = consts.tile([96, 3], _F32)
            pts = [ps.tile([128, 512], _F32, name=f"pt{b}", tag=f"pt{b}")
                   for b in range(3)]

            def emit_consts():
                nc.sync.dma_start(w8t[:], w8.ap())
                nc.sync.dma_start(w16t[:], w16.ap())
                nc.sync.dma_start(keep_s[:], keepf.ap())
                nc.vector.memset(out_t[:], 0)

            def emit_epilogue(r):
                saved_prio = tc.cur_priority
                tc.cur_priority = 5_000_000 + r * 10
                _emit_epilogue_inner(r)
                tc.cur_priority = saved_prio

            def _emit_epilogue_inner(r):
                reg = regions[r]
                ns, b, qb = reg['nslots'], reg['bank'], reg['qb']
                if ns == 0:
                    return
                c0 = b * 128
                lz = logz[qb:qb + ns, c0:c0 + 128]
                nc.scalar.activation(lz, pts[b][qb:qb + ns, 0:128],
                                     mybir.ActivationFunctionType.Ln)
                d = dum[qb:qb + ns, c0:c0 + 128]
                nc.vector.tensor_tensor(
                    d, lz, keep_s[qb:qb + ns, c0:c0 + 128],
                    op=mybir.AluOpType.mult)
                nc.vector.tensor_scalar(
                    d, d, 1.0, 0.0,
                    op0=mybir.AluOpType.mult, op1=mybir.AluOpType.add,
                    accum_out=out_t[qb:qb + ns, b:b + 1])

            def emit_mms(g, src, k):
                """src: fp8 exp tile [NP_,n,NT,M] (A) or int16 tile (F/P)."""
                r, q = info[g]
                reg = regions[r]
                if types[g] == 'A':
                    dst = pts[reg['bank']][0:32, 0:128]
                    for tp in range(4):
                        nc.tensor.matmul(
                            dst, w8t[:, q], src[:, k, 2 * tp:2 * tp + 2, :],
                            start=(mm_done[r] == 0),
                            stop=(mm_done[r] == mm_total[r] - 1),
                            perf_mode=mybir.MatmulPerfMode.DoubleRow,
                            skip_group_check=True)
                        mm_done[r] += 1
                else:
                    qb = reg['qb']
                    dst = pts[reg['bank']][qb:qb + 32, 0:128]
                    b16 = src.bitcast(_F16)
                    for t in range(NT):
                        nc.tensor.matmul(
                            dst, w16t[:, q], b16[:, k, t, :],
                            start=(mm_done[r] == 0),
                            stop=(mm_done[r] == mm_total[r] - 1),
                            skip_group_check=True)
                        mm_done[r] += 1
                if mm_done[r] == mm_total[r]:
                    closed.append((r, cur_ui[0]))

            closed = []
            epi_done = 0
            cur_ui = [0]

            def flush_epilogues(upto, min_age=0):
                nonlocal epi_done
                while epi_done < min(upto, len(closed)):
                    r, cui = closed[epi_done]
                    if min_age and cur_ui[0] < cui + min_age:
                        break
                    emit_epilogue(r)
                    epi_done += 1

            pools = {'A': (axp, e8p), 'F': (fxp, btp), 'P': (pxp, ptp)}
            pend = []          # (due_ui, g, ot, k)

            def flush_mms(ui):
                while pend and pend[0][0] <= ui:
                    _, g_, ot_, k_ = pend.pop(0)
                    emit_mms(g_, ot_, k_)

            g0 = 0
            for ui, (ut, ng) in enumerate(units):
                xpool, opool = pools[ut]
                xt = xpool.tile([NP_, 4, NT, M], _F8, tag="x")
                nc.sync.dma_start(
                    xt[:, 0:ng], x.ap()[:, g0 * 1024:(g0 + ng) * 1024])
                if ui == 7:
                    emit_consts()
                if ut == 'A':
                    ot = opool.tile([NP_, 4, NT, M], _F8, tag="o")
                    nc.scalar.activation(
                        ot[:, 0:ng], xt[:, 0:ng],
                        mybir.ActivationFunctionType.Exp)
                elif ut == 'F':
                    ot = opool.tile([NP_, 4, NT, M], _I16, tag="o")
                    nc.vector.tensor_scalar(
                        ot[:, 0:ng], xt[:, 0:ng], A_SCH, B_SCH,
                        op0=mybir.AluOpType.mult, op1=mybir.AluOpType.add)
                else:
                    ot = opool.tile([NP_, 4, NT, M], _I16, tag="o")
                    nc.gpsimd.tensor_scalar(
                        ot[:, 0:ng], xt[:, 0:ng], A_SCH, B_SCH,
                        op0=mybir.AluOpType.mult, op1=mybir.AluOpType.add)
                due = ui + (4 if ut == 'P' else 1)
                for k in range(ng):
                    pend.append((due, g0 + k, ot[:], k))
                pend.sort(key=lambda e: e[0])
                cur_ui[0] = ui
                flush_mms(ui)
                g0 += ng
            cur_ui[0] = 10 ** 9
            flush_mms(10 ** 9)
            flush_epilogues(9)

            nc.sync.dma_start(out.ap(), out_t[:])

    nc.compile()
    return nc


def kernel(cls_score, label, min_classes):
    cls_score = np.ascontiguousarray(np.asarray(cls_score, dtype=np.float32))
    label = np.asarray(label).astype(np.int64)
    min_classes = np.asarray(min_classes)

    keep = ~np.isin(label, min_classes)
    kept = np.nonzero(keep)[0]
    if kept.size == 0:
        return np.array(0.0, dtype=np.float32)

    per_core = -(-kept.size // NCORES)
    nsg = -(-per_core // M)
    nsg = -(-nsg // 4) * 4
    a_n, p_n = _pick_an_pn(nsg)
    assert nsg <= 288, f"row count needs more PSUM regions: {nsg}"
    cap = nsg * M

    global _cached_nc
    key = (nsg, a_n, p_n)
    nc = _cached.get(key)
    if nc is None:
        nc = _cached[key] = _build_nc(nsg, a_n, p_n)
    _cached_nc = nc

    _, types, info, regions = _sched(nsg, a_n, p_n)
    g_part = np.empty(nsg, dtype=np.int64)
    g_col = np.empty(nsg, dtype=np.int64)
    for g in range(nsg):
        r, q = info[g]
        g_part[g] = regions[r]['qb'] + q
        g_col[g] = regions[r]['bank'] * 128

    w8 = np.zeros((NP_, 32, 2, 32), dtype=ml_dtypes.float8_e4m3)
    w16 = np.zeros((NP_, 32, 32), dtype=np.float16)
    for q in range(32):
        w8[:, q, :, q] = 1.0
        w16[:, q, q] = 1.0

    in_maps = []
    gk_host = 0.0
    for s in range(NCORES):
        idx = kept[s * per_core:(s + 1) * per_core]
        n_c = idx.size
        xs = np.zeros((cap, C), dtype=np.float32)
        xs[:n_c] = cls_score[idx]
        np.clip(xs, -CLIP, CLIP, out=xs)
        gk_host += float(
            cls_score[idx, label[idx]].astype(np.float64).sum())

        xq = xs.astype(ml_dtypes.float8_e4m3)
        xr = np.ascontiguousarray(
            xq.reshape(nsg, M, NT, NP_).transpose(3, 0, 2, 1)
        ).reshape(NP_, nsg * 1024)

        ks = np.zeros(cap, dtype=np.float32)
        ks[:n_c] = 1.0
        keepf = np.zeros((96, 384), dtype=np.float32)
        ks2 = ks.reshape(nsg, M)
        for g in range(nsg):
            keepf[g_part[g], g_col[g]:g_col[g] + M] = ks2[g]

        in_maps.append({"x": xr, "w8": w8, "w16": w16, "keepf": keepf})

    results = run_bass_kernel_spmd(nc, in_maps, core_ids=list(range(NCORES)))
    parts = np.stack([np.asarray(r["out"]) for r in results.results])
    ce = parts.astype(np.float64).sum()
    loss = LOSS_WEIGHT * (ce - gk_host) / max(float(kept.size), 1.0)
    if not np.isfinite(loss):   # transient device fault guard: retry once
        results = run_bass_kernel_spmd(
            nc, in_maps, core_ids=list(range(NCORES)))
        parts = np.stack([np.asarray(r["out"]) for r in results.results])
        ce = parts.astype(np.float64).sum()
        loss = LOSS_WEIGHT * (ce - gk_host) / max(float(kept.size), 1.0)
    return np.array(loss, dtype=np.float32)


# revision 29
# speedup vs baseline: 1.0023x; 1.0023x over previous
"""CRLLoss (majority-masked mean CE) on 8 trn2 NeuronCores — v8 (PE reduce,
3-way exp split).

All-fp8 upload in class-major layout (classes on 125 SBUF partitions x 8
tiles, c = t*125 + p); rows padded to NSG supergroups of 128. Per supergroup
the device computes exp elementwise and reduces over classes with one-hot
stationary matmuls into PSUM, then Ln + keep-masked accumulation per PSUM
region. The loss denominator and the gathered x[label] sum are host-side
(the gather already is).

Three exp paths, balanced so every engine finishes with the DMA stream:
  A: ScalarE activation exp fp8->fp8, 4 DoubleRow fp8 matmuls (PSUM
     partitions [0:32) — DoubleRow cannot use output quadrant packing).
  F: DVE Schraudolph fp8->int16 (y = round(1477.32x + 15300.7), bitcast
     fp16), 8 fp16 one-hot matmuls into partitions [32:96).
  P: same as F but the tensor_scalar runs on GPSIMD (bit-identical on HW).

Emission: first 4 groups [A,F,A,F] with per-group DMAs (engine ramp-up),
then homogeneous runs of 4 groups (A/F/P Bresenham-interleaved, one DMA per
2 runs), last run per-group again to shorten the tail.
"""

import numpy as np
import ml_dtypes

import concourse.bass as bass
import concourse.tile as tile
from concourse import bacc, mybir
from concourse.bass_utils import run_bass_kernel_spmd

LOSS_WEIGHT = 1.0

N, C = 262144, 1000
NCORES = 8
NP_ = 125
NT = 8
M = 128
A_SCH = 1477.3196
B_SCH = 15300.7
CLIP = 5.5

_F32 = mybir.dt.float32
_F16 = mybir.dt.float16
_F8 = mybir.dt.float8e4
_I16 = mybir.dt.int16

_cached = {}
_cached_nc = None


def _pick_an_pn(nsg):
    a_n = max(8, min(92, int(round(nsg * 0.36 / 4)) * 4, nsg - 8))
    return a_n, 0


TAPER = [('A', 2), ('F', 2), ('A', 2), ('F', 2),
         ('A', 1), ('F', 1), ('A', 1), ('F', 1)]


def _sched(nsg, a_n, p_n):
    """units [(type, ngroups)]; types[g]; info[g] = (region, q); regions."""
    assert nsg % 4 == 0 and nsg >= 32 and a_n % 4 == 0 and p_n % 4 == 0
    f_n = nsg - a_n - p_n
    assert 8 <= a_n <= 96 and (f_n + p_n) <= 192 and f_n >= 8

    n_mid = (nsg - 16) // 4
    aruns = (a_n - 8) // 4
    pruns = p_n // 4
    nb_a = n_mid - 2           # no A-units in the last 2 mid positions
    mid = []
    ca = 0
    for i in range(n_mid):
        if i < nb_a and ((i + 1) * aruns) // nb_a > ca:
            mid.append('A')
            ca += 1
        else:
            mid.append('F')
    # place P among F positions, excluding the last few (Pool latency)
    fpos = [i for i, t in enumerate(mid) if t == 'F' and i < n_mid - 4]
    cp = 0
    for j in range(len(fpos)):
        if ((j + 1) * pruns) // len(fpos) > cp:
            mid[fpos[j]] = 'P'
            cp += 1

    units = [('A', 1), ('F', 1), ('A', 1), ('F', 1)]
    units += [(t, 4) for t in mid]
    units += TAPER

    types = []
    for t, n in units:
        types += [t] * n
    assert len(types) == nsg and types.count('A') == a_n

    regions = []
    for b in range(3):
        regions.append(dict(kind='A', bank=b, qb=0,
                            nslots=max(0, min(32, a_n - 32 * b))))
    fp_n = f_n + p_n
    for rf in range(6):
        regions.append(dict(kind='F', bank=rf // 2, qb=32 * (1 + rf % 2),
                            nslots=max(0, min(32, fp_n - 32 * rf))))

    info = []
    ai = fi = 0
    for g in range(nsg):
        if types[g] == 'A':
            info.append((ai // 32, ai % 32))
            ai += 1
        else:
            info.append((3 + fi // 32, fi % 32))
            fi += 1
    return units, types, info, regions


def _build_nc(nsg, a_n, p_n):
    units, types, info, regions = _sched(nsg, a_n, p_n)
    nc = bacc.Bacc("TRN2", debug=False, target_bir_lowering=False)

    x = nc.dram_tensor("x", [NP_, nsg * 1024], _F8, kind="ExternalInput")
    w8 = nc.dram_tensor("w8", [NP_, 32, 2, 32], _F8, kind="ExternalInput")
    w16 = nc.dram_tensor("w16", [NP_, 32, 32], _F16, kind="ExternalInput")
    keepf = nc.dram_tensor("keepf", [96, 384], _F32, kind="ExternalInput")
    out = nc.dram_tensor("out", [96, 3], _F32, kind="ExternalOutput")

    mm_total = [regions[r]['nslots'] * (4 if regions[r]['kind'] == 'A' else 8)
                for r in range(9)]
    mm_done = [0] * 9

    with tile.TileContext(nc) as tc:
        with (
            tc.tile_pool(name="axp", bufs=5) as axp,
            tc.tile_pool(name="fxp", bufs=5) as fxp,
            tc.tile_pool(name="pxp", bufs=6) as pxp,
            tc.tile_pool(name="e8p", bufs=4) as e8p,
            tc.tile_pool(name="btp", bufs=3) as btp,
            tc.tile_pool(name="ptp", bufs=6) as ptp,
            tc.tile_pool(name="consts", bufs=1) as consts,
            tc.tile_pool(name="ps", bufs=1, space="PSUM") as ps,
        ):
            nc.scalar.add_instruction(mybir.InstLoadActFuncSet(
                name=nc.get_next_instruction_name(), ins=[], outs=[],
                act_func_set_id=6))

            w8t = consts.tile([NP_, 32, 2, 32], _F8)
            w16t = consts.tile([NP_, 32, 32], _F16)
            keep_s = consts.tile([96, 384], _F32)
            logz = consts.tile([96, 384], _F32)
            dum = consts.tile([96, 384], _F32)
            out_t = consts.tile([96, 3], _F32)
            pts = [ps.tile([128, 512], _F32, name=f"pt{b}", tag=f"pt{b}")
                   for b in range(3)]

            def emit_consts():
                nc.sync.dma_start(w8t[:], w8.ap())
                nc.sync.dma_start(w16t[:], w16.ap())
                nc.sync.dma_start(keep_s[:], keepf.ap())
                nc.vector.memset(out_t[:], 0)

            def emit_epilogue(r):
                saved_prio = tc.cur_priority
                tc.cur_priority = 5_000_000 + r * 10
                _emit_epilogue_inner(r)
                tc.cur_priority = saved_prio

            def _emit_epilogue_inner(r):
                reg = regions[r]
                ns, b, qb = reg['nslots'], reg['bank'], reg['qb']
                if ns == 0:
                    return
                c0 = b * 128
                lz = logz[qb:qb + ns, c0:c0 + 128]
                nc.scalar.activation(lz, pts[b][qb:qb + ns, 0:128],
                                     mybir.ActivationFunctionType.Ln)
                d = dum[qb:qb + ns, c0:c0 + 128]
                nc.vector.scalar_tensor_tensor(
                    d, lz, 1.0, keep_s[qb:qb + ns, c0:c0 + 128],
                    op0=mybir.AluOpType.mult, op1=mybir.AluOpType.mult,
                    accum_out=out_t[qb:qb + ns, b:b + 1])

            def emit_mms(g, src, k):
                """src: fp8 exp tile [NP_,n,NT,M] (A) or int16 tile (F/P)."""
                r, q = info[g]
                reg = regions[r]
                if types[g] == 'A':
                    dst = pts[reg['bank']][0:32, 0:128]
                    for tp in range(4):
                        nc.tensor.matmul(
                            dst, w8t[:, q], src[:, k, 2 * tp:2 * tp + 2, :],
                            start=(mm_done[r] == 0),
                            stop=(mm_done[r] == mm_total[r] - 1),
                            perf_mode=mybir.MatmulPerfMode.DoubleRow,
                            skip_group_check=True)
                        mm_done[r] += 1
                else:
                    qb = reg['qb']
                    dst = pts[reg['bank']][qb:qb + 32, 0:128]
                    b16 = src.bitcast(_F16)
                    for t in range(NT):
                        nc.tensor.matmul(
                            dst, w16t[:, q], b16[:, k, t, :],
                            start=(mm_done[r] == 0),
                            stop=(mm_done[r] == mm_total[r] - 1),
                            skip_group_check=True)
                        mm_done[r] += 1
                if mm_done[r] == mm_total[r]:
                    closed.append((r, cur_ui[0]))

            closed = []
            epi_done = 0
            cur_ui = [0]

            def flush_epilogues(upto, min_age=0):
                nonlocal epi_done
                while epi_done < min(upto, len(closed)):
                    r, cui = closed[epi_done]
                    if min_age and cur_ui[0] < cui + min_age:
                        break
                    emit_epilogue(r)
                    epi_done += 1

            pools = {'A': (axp, e8p), 'F': (fxp, btp), 'P': (pxp, ptp)}
            pend = []          # (due_ui, g, ot, k)

            def flush_mms(ui):
                while pend and pend[0][0] <= ui:
                    _, g_, ot_, k_ = pend.pop(0)
                    emit_mms(g_, ot_, k_)

            g0 = 0
            for ui, (ut, ng) in enumerate(units):
                xpool, opool = pools[ut]
                xt = xpool.tile([NP_, 4, NT, M], _F8, tag="x")
                nc.sync.dma_start(
                    xt[:, 0:ng], x.ap()[:, g0 * 1024:(g0 + ng) * 1024])
                if ui == 7:
                    emit_consts()
                if ut == 'A':
                    ot = opool.tile([NP_, 4, NT, M], _F8, tag="o")
                    nc.scalar.activation(
                        ot[:, 0:ng], xt[:, 0:ng],
                        mybir.ActivationFunctionType.Exp)
                elif ut == 'F':
                    ot = opool.tile([NP_, 4, NT, M], _I16, tag="o")
                    nc.vector.tensor_scalar(
                        ot[:, 0:ng], xt[:, 0:ng], A_SCH, B_SCH,
                        op0=mybir.AluOpType.mult, op1=mybir.AluOpType.add)
                else:
                    ot = opool.tile([NP_, 4, NT, M], _I16, tag="o")
                    nc.gpsimd.tensor_scalar(
                        ot[:, 0:ng], xt[:, 0:ng], A_SCH, B_SCH,
                        op0=mybir.AluOpType.mult, op1=mybir.AluOpType.add)
                due = ui + (4 if ut == 'P' else 1)
                for k in range(ng):
                    pend.append((due, g0 + k, ot[:], k))
                pend.sort(key=lambda e: e[0])
                cur_ui[0] = ui
                flush_mms(ui)
                g0 += ng
            cur_ui[0] = 10 ** 9
            flush_mms(10 ** 9)
            flush_epilogues(9)

            nc.sync.dma_start(out.ap(), out_t[:])

    nc.compile()
    return nc


def kernel(cls_score, label, min_classes):
    cls_score = np.ascontiguousarray(np.asarray(cls_score, dtype=np.float32))
    label = np.asarray(label).astype(np.int64)
    min_classes = np.asarray(min_classes)

    keep = ~np.isin(label, min_classes)
    kept = np.nonzero(keep)[0]
    if kept.size == 0:
        return np.array(0.0, dtype=np.float32)

    per_core = -(-kept.size // NCORES)
    nsg = -(-per_core // M)
    nsg = -(-nsg // 4) * 4
    a_n, p_n = _pick_an_pn(nsg)
    assert nsg <= 288, f"row count needs more PSUM regions: {nsg}"
    cap = nsg * M

    global _cached_nc
    key = (nsg, a_n, p_n)
    nc = _cached.get(key)
    if nc is None:
        nc = _cached[key] = _build_nc(nsg, a_n, p_n)
    _cached_nc = nc

    _, types, info, regions = _sched(nsg, a_n, p_n)
    g_part = np.empty(nsg, dtype=np.int64)
    g_col = np.empty(nsg, dtype=np.int64)
    for g in range(nsg):
        r, q = info[g]
        g_part[g] = regions[r]['qb'] + q
        g_col[g] = regions[r]['bank'] * 128

    w8 = np.zeros((NP_, 32, 2, 32), dtype=ml_dtypes.float8_e4m3)
    w16 = np.zeros((NP_, 32, 32), dtype=np.float16)
    for q in range(32):
        w8[:, q, :, q] = 1.0
        w16[:, q, q] = 1.0

    in_maps = []
    gk_host = 0.0
    for s in range(NCORES):
        idx = kept[s * per_core:(s + 1) * per_core]
        n_c = idx.size
        xs = np.zeros((cap, C), dtype=np.float32)
        xs[:n_c] = cls_score[idx]
        np.clip(xs, -CLIP, CLIP, out=xs)
        gk_host += float(
            cls_score[idx, label[idx]].astype(np.float64).sum())

        xq = xs.astype(ml_dtypes.float8_e4m3)
        xr = np.ascontiguousarray(
            xq.reshape(nsg, M, NT, NP_).transpose(3, 0, 2, 1)
        ).reshape(NP_, nsg * 1024)

        ks = np.zeros(cap, dtype=np.float32)
        ks[:n_c] = 1.0
        keepf = np.zeros((96, 384), dtype=np.float32)
        ks2 = ks.reshape(nsg, M)
        for g in range(nsg):
            keepf[g_part[g], g_col[g]:g_col[g] + M] = ks2[g]

        in_maps.append({"x": xr, "w8": w8, "w16": w16, "keepf": keepf})

    results = run_bass_kernel_spmd(nc, in_maps, core_ids=list(range(NCORES)))
    parts = np.stack([np.asarray(r["out"]) for r in results.results])
    ce = parts.astype(np.float64).sum()
    loss = LOSS_WEIGHT * (ce - gk_host) / max(float(kept.size), 1.0)
    if not np.isfinite(loss):   # transient device fault guard: retry once
        results = run_bass_kernel_spmd(
            nc, in_maps, core_ids=list(range(NCORES)))
        parts = np.stack([np.asarray(r["out"]) for r in results.results])
        ce = parts.astype(np.float64).sum()
        loss = LOSS_WEIGHT * (ce - gk_host) / max(float(kept.size), 1.0)
    return np.array(loss, dtype=np.float32)


# revision 33
# speedup vs baseline: 1.0135x; 1.0112x over previous
"""CRLLoss (majority-masked mean CE) on 8 trn2 NeuronCores — v8 (PE reduce,
3-way exp split).

All-fp8 upload in class-major layout (classes on 125 SBUF partitions x 8
tiles, c = t*125 + p); rows padded to NSG supergroups of 128. Per supergroup
the device computes exp elementwise and reduces over classes with one-hot
stationary matmuls into PSUM, then Ln + keep-masked accumulation per PSUM
region. The loss denominator and the gathered x[label] sum are host-side
(the gather already is).

Three exp paths, balanced so every engine finishes with the DMA stream:
  A: ScalarE activation exp fp8->fp8, 4 DoubleRow fp8 matmuls (PSUM
     partitions [0:32) — DoubleRow cannot use output quadrant packing).
  F: DVE Schraudolph fp8->int16 (y = round(1477.32x + 15300.7), bitcast
     fp16), 8 fp16 one-hot matmuls into partitions [32:96).
  P: same as F but the tensor_scalar runs on GPSIMD (bit-identical on HW).

Emission: first 4 groups [A,F,A,F] with per-group DMAs (engine ramp-up),
then homogeneous runs of 4 groups (A/F/P Bresenham-interleaved, one DMA per
2 runs), last run per-group again to shorten the tail.
"""

import numpy as np
import ml_dtypes

import concourse.bass as bass
import concourse.tile as tile
from concourse import bacc, mybir
from concourse.bass_utils import run_bass_kernel_spmd

LOSS_WEIGHT = 1.0

N, C = 262144, 1000
NCORES = 8
NP_ = 125
NT = 8
M = 128
A_SCH = 1477.3196
B_SCH = 15300.7
CLIP = 5.5

_F32 = mybir.dt.float32
_F16 = mybir.dt.float16
_F8 = mybir.dt.float8e4
_I16 = mybir.dt.int16

_cached = {}
_cached_nc = None


def _pick_an_pn(nsg):
    a_n = max(8, min(92, int(round(nsg * 0.36 / 4)) * 4, nsg - 8))
    return a_n, 0


def _sched(nsg, a_n, p_n):
    """units [(type, ngroups)]; types[g]; info[g] = (region, q); regions."""
    assert nsg % 4 == 0 and nsg >= 32 and a_n % 4 == 0 and p_n % 4 == 0
    f_n = nsg - a_n - p_n
    assert 8 <= a_n <= 96 and (f_n + p_n) <= 192 and f_n >= 8

    n_mid = (nsg - 16) // 4
    aruns = (a_n - 8) // 4
    pruns = p_n // 4
    nb_a = n_mid - 2           # no A-units in the last 2 mid positions
    mid = []
    ca = 0
    for i in range(n_mid):
        if i < nb_a and ((i + 1) * aruns) // nb_a > ca:
            mid.append('A')
            ca += 1
        else:
            mid.append('F')
    # place P among F positions, excluding the last few (Pool latency)
    fpos = [i for i, t in enumerate(mid) if t == 'F' and i < n_mid - 4]
    cp = 0
    for j in range(len(fpos)):
        if ((j + 1) * pruns) // len(fpos) > cp:
            mid[fpos[j]] = 'P'
            cp += 1

    units = [('A', 1), ('F', 1), ('A', 1), ('F', 1)]
    units += [(t, 4) for t in mid]
    units += [('A', 2), ('F', 2), ('A', 2), ('F', 2),
              ('A', 1), ('F', 1), ('A', 1), ('F', 1)]

    types = []
    for t, n in units:
        types += [t] * n
    assert len(types) == nsg and types.count('A') == a_n

    regions = []
    for b in range(3):
        regions.append(dict(kind='A', bank=b, qb=0,
                            nslots=max(0, min(32, a_n - 32 * b))))
    fp_n = f_n + p_n
    for rf in range(6):
        regions.append(dict(kind='F', bank=rf // 2, qb=32 * (1 + rf % 2),
                            nslots=max(0, min(32, fp_n - 32 * rf))))

    info = []
    ai = fi = 0
    for g in range(nsg):
        if types[g] == 'A':
            info.append((ai // 32, ai % 32))
            ai += 1
        else:
            info.append((3 + fi // 32, fi % 32))
            fi += 1
    return units, types, info, regions


def _build_nc(nsg, a_n, p_n):
    units, types, info, regions = _sched(nsg, a_n, p_n)
    nc = bacc.Bacc("TRN2", debug=False, target_bir_lowering=False)

    x = nc.dram_tensor("x", [NP_, nsg * 1024], _F8, kind="ExternalInput")
    w8 = nc.dram_tensor("w8", [NP_, 32, 2, 32], _F8, kind="ExternalInput")
    w16 = nc.dram_tensor("w16", [NP_, 32, 32], _F16, kind="ExternalInput")
    keepf = nc.dram_tensor("keepf", [96, 384], _F32, kind="ExternalInput")
    out = nc.dram_tensor("out", [96, 3], _F32, kind="ExternalOutput")

    mm_total = [regions[r]['nslots'] * (4 if regions[r]['kind'] == 'A' else 8)
                for r in range(9)]
    mm_done = [0] * 9

    with tile.TileContext(nc) as tc:
        with (
            tc.tile_pool(name="axp", bufs=5) as axp,
            tc.tile_pool(name="fxp", bufs=5) as fxp,
            tc.tile_pool(name="pxp", bufs=6) as pxp,
            tc.tile_pool(name="e8p", bufs=4) as e8p,
            tc.tile_pool(name="btp", bufs=3) as btp,
            tc.tile_pool(name="ptp", bufs=6) as ptp,
            tc.tile_pool(name="consts", bufs=1) as consts,
            tc.tile_pool(name="ps", bufs=1, space="PSUM") as ps,
        ):
            nc.scalar.add_instruction(mybir.InstLoadActFuncSet(
                name=nc.get_next_instruction_name(), ins=[], outs=[],
                act_func_set_id=6))

            w8t = consts.tile([NP_, 32, 2, 32], _F8)
            w16t = consts.tile([NP_, 32, 32], _F16)
            keep_s = consts.tile([96, 384], _F32)
            logz = consts.tile([96, 384], _F32)
            dum = consts.tile([96, 384], _F32)
            out_t = consts.tile([96, 3], _F32)
            pts = [ps.tile([128, 512], _F32, name=f"pt{b}", tag=f"pt{b}")
                   for b in range(3)]

            def emit_consts():
                nc.sync.dma_start(w8t[:], w8.ap())
                nc.sync.dma_start(w16t[:], w16.ap())
                nc.sync.dma_start(keep_s[:], keepf.ap())
                nc.vector.memset(out_t[:], 0)

            def emit_epilogue(r):
                saved_prio = tc.cur_priority
                tc.cur_priority = 5_000_000 + r * 10
                _emit_epilogue_inner(r)
                tc.cur_priority = saved_prio

            def _emit_epilogue_inner(r):
                reg = regions[r]
                ns, b, qb = reg['nslots'], reg['bank'], reg['qb']
                if ns == 0:
                    return
                c0 = b * 128
                lz = logz[qb:qb + ns, c0:c0 + 128]
                nc.scalar.activation(lz, pts[b][qb:qb + ns, 0:128],
                                     mybir.ActivationFunctionType.Ln)
                d = dum[qb:qb + ns, c0:c0 + 128]
                nc.vector.scalar_tensor_tensor(
                    d, lz, 1.0, keep_s[qb:qb + ns, c0:c0 + 128],
                    op0=mybir.AluOpType.mult, op1=mybir.AluOpType.mult,
                    accum_out=out_t[qb:qb + ns, b:b + 1])

            def emit_mms(g, src, k):
                """src: fp8 exp tile [NP_,n,NT,M] (A) or int16 tile (F/P)."""
                r, q = info[g]
                reg = regions[r]
                if types[g] == 'A':
                    dst = pts[reg['bank']][0:32, 0:128]
                    for tp in range(4):
                        nc.tensor.matmul(
                            dst, w8t[:, q], src[:, k, 2 * tp:2 * tp + 2, :],
                            start=(mm_done[r] == 0),
                            stop=(mm_done[r] == mm_total[r] - 1),
                            perf_mode=mybir.MatmulPerfMode.DoubleRow,
                            skip_group_check=True)
                        mm_done[r] += 1
                else:
                    qb = reg['qb']
                    dst = pts[reg['bank']][qb:qb + 32, 0:128]
                    b16 = src.bitcast(_F16)
                    for t in range(NT):
                        nc.tensor.matmul(
                            dst, w16t[:, q], b16[:, k, t, :],
                            start=(mm_done[r] == 0),
                            stop=(mm_done[r] == mm_total[r] - 1),
                            skip_group_check=True)
                        mm_done[r] += 1
                if mm_done[r] == mm_total[r]:
                    closed.append((r, cur_ui[0]))

            closed = []
            epi_done = 0
            cur_ui = [0]

            def flush_epilogues(upto, min_age=0):
                nonlocal epi_done
                while epi_done < min(upto, len(closed)):
                    r, cui = closed[epi_done]
                    if min_age and cur_ui[0] < cui + min_age:
                        break
                    emit_epilogue(r)
                    epi_done += 1

            pools = {'A': (axp, e8p), 'F': (fxp, btp), 'P': (pxp, ptp)}
            pend = []          # (due_ui, g, ot, k)

            def flush_mms(ui):
                while pend and pend[0][0] <= ui:
                    _, g_, ot_, k_ = pend.pop(0)
                    emit_mms(g_, ot_, k_)

            g0 = 0
            hxt = None
            for ui, (ut, ng) in enumerate(units):
                xpool, opool = pools[ut]
                if ui < 4:
                    # head: pairs (A,F) share one 2-group DMA
                    if ui % 2 == 0:
                        hxt = xpool.tile([NP_, 4, NT, M], _F8, tag="hx")
                        nc.sync.dma_start(
                            hxt[:, 0:2],
                            x.ap()[:, g0 * 1024:(g0 + 2) * 1024])
                        xt = hxt
                        hoff = 0
                    else:
                        xt = hxt
                        hoff = 1
                    xsl = xt[:, hoff:hoff + 1]
                else:
                    xt = xpool.tile([NP_, 4, NT, M], _F8, tag="x")
                    nc.sync.dma_start(
                        xt[:, 0:ng], x.ap()[:, g0 * 1024:(g0 + ng) * 1024])
                    xsl = xt[:, 0:ng]
                if ui == 7:
                    emit_consts()
                if ut == 'A':
                    ot = opool.tile([NP_, 4, NT, M], _F8, tag="o")
                    nc.scalar.activation(
                        ot[:, 0:ng], xsl,
                        mybir.ActivationFunctionType.Exp)
                elif ut == 'F':
                    ot = opool.tile([NP_, 4, NT, M], _I16, tag="o")
                    nc.vector.tensor_scalar(
                        ot[:, 0:ng], xsl, A_SCH, B_SCH,
                        op0=mybir.AluOpType.mult, op1=mybir.AluOpType.add)
                else:
                    ot = opool.tile([NP_, 4, NT, M], _I16, tag="o")
                    nc.gpsimd.tensor_scalar(
                        ot[:, 0:ng], xsl, A_SCH, B_SCH,
                        op0=mybir.AluOpType.mult, op1=mybir.AluOpType.add)
                due = ui + (4 if ut == 'P' else 1)
                for k in range(ng):
                    pend.append((due, g0 + k, ot[:], k))
                pend.sort(key=lambda e: e[0])
                cur_ui[0] = ui
                flush_mms(ui)
                g0 += ng
            cur_ui[0] = 10 ** 9
            flush_mms(10 ** 9)
            flush_epilogues(9)

            nc.sync.dma_start(out.ap(), out_t[:])

    nc.compile()
    return nc


def kernel(cls_score, label, min_classes):
    cls_score = np.ascontiguousarray(np.asarray(cls_score, dtype=np.float32))
    label = np.asarray(label).astype(np.int64)
    min_classes = np.asarray(min_classes)

    keep = ~np.isin(label, min_classes)
    kept = np.nonzero(keep)[0]
    if kept.size == 0:
        return np.array(0.0, dtype=np.float32)

    per_core = -(-kept.size // NCORES)
    nsg = -(-per_core // M)
    nsg = -(-nsg // 4) * 4
    a_n, p_n = _pick_an_pn(nsg)
    assert nsg <= 288, f"row count needs more PSUM regions: {nsg}"
    cap = nsg * M

    global _cached_nc
    key = (nsg, a_n, p_n)
    nc = _cached.get(key)
    if nc is None:
        nc = _cached[key] = _build_nc(nsg, a_n, p_n)
    _cached_nc = nc

    _, types, info, regions = _sched(nsg, a_n, p_n)
    g_part = np.empty(nsg, dtype=np.int64)
    g_col = np.empty(nsg, dtype=np.int64)
    for g in range(nsg):
        r, q = info[g]
        g_part[g] = regions[r]['qb'] + q
        g_col[g] = regions[r]['bank'] * 128

    w8 = np.zeros((NP_, 32, 2, 32), dtype=ml_dtypes.float8_e4m3)
    w16 = np.zeros((NP_, 32, 32), dtype=np.float16)
    for q in range(32):
        w8[:, q, :, q] = 1.0
        w16[:, q, q] = 1.0

    in_maps = []
    gk_host = 0.0
    for s in range(NCORES):
        idx = kept[s * per_core:(s + 1) * per_core]
        n_c = idx.size
        xs = np.zeros((cap, C), dtype=np.float32)
        xs[:n_c] = cls_score[idx]
        np.clip(xs, -CLIP, CLIP, out=xs)
        gk_host += float(
            cls_score[idx, label[idx]].astype(np.float64).sum())

        xq = xs.astype(ml_dtypes.float8_e4m3)
        xr = np.ascontiguousarray(
            xq.reshape(nsg, M, NT, NP_).transpose(3, 0, 2, 1)
        ).reshape(NP_, nsg * 1024)

        ks = np.zeros(cap, dtype=np.float32)
        ks[:n_c] = 1.0
        keepf = np.zeros((96, 384), dtype=np.float32)
        ks2 = ks.reshape(nsg, M)
        for g in range(nsg):
            keepf[g_part[g], g_col[g]:g_col[g] + M] = ks2[g]

        in_maps.append({"x": xr, "w8": w8, "w16": w16, "keepf": keepf})

    results = run_bass_kernel_spmd(nc, in_maps, core_ids=list(range(NCORES)))
    parts = np.stack([np.asarray(r["out"]) for r in results.results])
    ce = parts.astype(np.float64).sum()
    loss = LOSS_WEIGHT * (ce - gk_host) / max(float(kept.size), 1.0)
    if not np.isfinite(loss):   # transient device fault guard: retry once
        results = run_bass_kernel_spmd(
            nc, in_maps, core_ids=list(range(NCORES)))
        parts = np.stack([np.asarray(r["out"]) for r in results.results])
        ce = parts.astype(np.float64).sum()
        loss = LOSS_WEIGHT * (ce - gk_host) / max(float(kept.size), 1.0)
    return np.array(loss, dtype=np.float32)
